# revision 3
# baseline (speedup 1.0000x reference)
"""MultiHeadLinearAttention Trainium2 Bass kernel — 8-core SPMD (v3, bf16).

Problem (per reference):
  q = elu(LN(Xq @ Wq.T + bq)) + 1 ; k = elu(LN(Xk @ Wk.T + bk)) + 1
  v = Xv @ Wv.T + bv
  vk = sum_n v[n] (x) k[n]   (per head, [D,D]);  ksum = sum_n k[n]
  out = ((q @ vk.T) / (q . ksum)) @ Wo.T + bo

Sharding: core c -> batch b = c//2, token half h = c%2 (2048 q AND k/v
tokens each). Per-pair AllReduce of vk/ksum partials (~266 KB).

v3 vs v2:
  - all matmul operands bf16 (same PE rate as f32r, half DMA/SBUF,
    2x DVE elementwise).
  - num matmul folded into out-proj: M = blockdiag(vk) @ Wo^T computed
    once post-AR; C consumes q~ = (elu(q)+1)/den directly.
  - phase-A LN rstd via quake-rsqrt on DVE (no ACT Ln -> no act-table
    thrash; only B1's batched Ln/Exp touches the table mid-kernel).
  - den via block-diag ksum lhsT accumulated into one [16,512] PSUM ->
    single reciprocal per chunk.
  - AR unpack = 16 same-base [64,64] copies (gpsimd) into zeroed
    block-diag vkbd; no strided rearranges.
  - weights/xq preloaded during phase A into non-aliasing SBUF.
"""

import numpy as np

B, NSEQ, E, H, D = 4, 4096, 1024, 16, 64
NCORES = 8
T = NSEQ // 2          # tokens per core
TT = T // 128          # token tiles (16)
EI = E // 128          # feature blocks (8)
S = 4                  # B-phase token chunks
SC = T // S            # 512 tokens per chunk
LN_EPS = 1e-5

_NC_CACHE = {}


def _build_nc(debug=False):
    from concourse import bacc
    import concourse.mybir as mybir
    import concourse.tile as tile

    f32 = mybir.dt.float32
    bf16 = mybir.dt.bfloat16
    i32 = mybir.dt.int32
    Alu = mybir.AluOpType
    Act = mybir.ActivationFunctionType
    RG = [[0, 1], [2, 3], [4, 5], [6, 7]]

    nc = bacc.Bacc(num_devices=NCORES)

    xqT = nc.dram_tensor("xqT", [E, T], bf16, kind="ExternalInput")
    xkT = nc.dram_tensor("xkT", [E, T], bf16, kind="ExternalInput")
    xvT = nc.dram_tensor("xvT", [E, T], bf16, kind="ExternalInput")
    wqT = nc.dram_tensor("wqT", [E, E], bf16, kind="ExternalInput")
    wkT = nc.dram_tensor("wkT", [E, E], bf16, kind="ExternalInput")
    wvT = nc.dram_tensor("wvT", [E, E], bf16, kind="ExternalInput")
    woT = nc.dram_tensor("woT", [E, E], bf16, kind="ExternalInput")
    bq2d = nc.dram_tensor("bq2d", [128, EI], f32, kind="ExternalInput")
    bkR = nc.dram_tensor("bkR", [1, E], f32, kind="ExternalInput")
    bvR = nc.dram_tensor("bvR", [1, E], f32, kind="ExternalInput")
    ebcR = nc.dram_tensor("ebcR", [16, EI * 128], bf16, kind="ExternalInput")
    out_d = nc.dram_tensor("out", [T, E], f32, kind="ExternalOutput")
    if debug:
        dbg = {
            "d_rstdA": nc.dram_tensor("d_rstdA", [128, TT], f32,
                                      kind="ExternalOutput"),
            "d_kf0": nc.dram_tensor("d_kf0", [128, E], bf16,
                                    kind="ExternalOutput"),
            "d_pack": nc.dram_tensor("d_pack", [128, 520], bf16,
                                     kind="ExternalOutput"),
            "d_ar": nc.dram_tensor("d_ar", [128, 520], bf16,
                                   kind="ExternalOutput"),
            "d_M": nc.dram_tensor("d_M", [128, EI * E], bf16,
                                  kind="ExternalOutput"),
            "d_rb0": nc.dram_tensor("d_rb0", [128, SC], bf16,
                                    kind="ExternalOutput"),
            "d_dinv0": nc.dram_tensor("d_dinv0", [16, 512], bf16,
                                      kind="ExternalOutput"),
            "d_qt00": nc.dram_tensor("d_qt00", [128, SC], bf16,
                                     kind="ExternalOutput"),
            "d_u00": nc.dram_tensor("d_u00", [128, SC], bf16,
                                    kind="ExternalOutput"),
        }

    xqT_v = xqT.rearrange("(i p) n -> p i n", p=128)
    xkT_v = xkT.rearrange("(i p) n -> p i n", p=128)
    xvT_v = xvT.rearrange("(i p) n -> p i n", p=128)
    wqT_v = wqT.rearrange("(i p) j -> p i j", p=128)
    wkT_v = wkT.rearrange("(i p) j -> p i j", p=128)
    wvT_v = wvT.rearrange("(i p) j -> p i j", p=128)
    woT_v = woT.rearrange("(e p) j -> p e j", p=128)

    QK3 = 0x5F3759DF + 1  # quake constant for ~x + (C+1) == C - (x>>1)

    with tile.TileContext(nc) as tc:
        with tc.tile_pool(name="const", bufs=1) as cp, \
             tc.tile_pool(name="dram", bufs=1, space="DRAM") as dp:
            # ---- long-lived constants and cross-phase tiles ----
            onesC = cp.tile([128, 1], bf16, tag="onesC")
            nc.vector.memset(onesC, 1.0)
            ones_row = cp.tile([1, 128], bf16, tag="ones_row")
            nc.vector.memset(ones_row, 1.0)
            eps_row = cp.tile([1, 1], f32, tag="eps_row")
            nc.vector.memset(eps_row, LN_EPS)
            bq_sb = cp.tile([128, EI], f32, tag="bq_sb")
            nc.scalar.dma_start(out=bq_sb, in_=bq2d[:, :])
            bk_b = cp.tile([128, E], f32, tag="bk_b")
            nc.scalar.dma_start(out=bk_b, in_=bkR[:, :].to_broadcast([128, E]))
            bv_b = cp.tile([128, E], f32, tag="bv_b")
            nc.scalar.dma_start(out=bv_b, in_=bvR[:, :].to_broadcast([128, E]))
            ebc = cp.tile([16, EI, 128], bf16, tag="ebc")
            nc.sync.dma_start(
                out=ebc, in_=ebcR.rearrange("h (j c) -> h j c", c=128))
            vkbd = cp.tile([128, EI * 128], bf16, tag="vkbd")
            nc.vector.memset(vkbd, 0.0)
            ks3 = cp.tile([128, EI, 16], bf16, tag="ks3")
            nc.vector.memset(ks3, 0.0)
            ar_sb = cp.tile([128, 520], bf16, tag="ar_sb")
            pack = cp.tile([128, 520], bf16, tag="pack")
            M_sb = cp.tile([128, EI, E], bf16, tag="M_sb")
            wq_sb = cp.tile([128, EI, E], bf16, tag="wq_sb")
            wo_sb = cp.tile([128, EI, E], bf16, tag="wo_sb")
            u_t = [[cp.tile([128, SC], bf16, tag=f"u{j}_{s}",
                            name=f"u{j}_{s}") for s in range(S)]
                   for j in range(EI)]
            rb_t = [cp.tile([128, SC], bf16, tag=f"rb{s}", name=f"rb{s}")
                    for s in range(S)]
            cc_in = dp.tile([128, 520], bf16, tag="cc_in")
            cc_out = dp.tile([128, 520], bf16, tag="cc_out")

            # ============ Phase A: k/v proj + elu + vk/ksum ============
            pbx_cm = tc.tile_pool(name="pbx", bufs=2)
            pbx = pbx_cm.__enter__()
            xq_tiles = []

            def load_xq(s):
                xq_s = pbx.tile([128, EI, SC], bf16, tag="xq")
                nc.sync.dma_start(out=xq_s,
                                  in_=xqT_v[:, :, slice(SC * s, SC * s + SC)])
                xq_tiles.append(xq_s)

            with tc.tile_pool(name="paw", bufs=1) as paw, \
                 tc.tile_pool(name="pax", bufs=2) as pax, \
                 tc.tile_pool(name="par", bufs=2) as par, \
                 tc.tile_pool(name="psK", bufs=3, space="PSUM") as psK, \
                 tc.tile_pool(name="psV", bufs=2, space="PSUM") as psV, \
                 tc.tile_pool(name="psA", bufs=1, space="PSUM") as psA:
                wk_sb = paw.tile([128, EI, E], bf16, tag="wk")
                wv_sb = paw.tile([128, EI, E], bf16, tag="wv")
                ss_all = paw.tile([128, TT], f32, tag="ss_all")
                rstd_all = paw.tile([128, TT], f32, tag="rstd_all")
                qa = paw.tile([128, TT], f32, tag="qa")
                qc = paw.tile([128, TT], f32, tag="qc")

                vk_ps = psA.tile([128, EI * 128], f32, tag="vkps")
                ksum_ps = psA.tile([128, EI], f32, tag="ksum")
                # one full-width start=True zero-matmul per accumulation
                # bank (start clears has_written BANK-wide, so per-region
                # start flags would drop earlier regions' first-tile data)
                zrow = paw.tile([1, 512], bf16, tag="zrow")
                nc.vector.memset(zrow, 0.0)
                for zh in range(2):
                    nc.tensor.matmul(vk_ps[:, 512 * zh:512 * zh + 512],
                                     ones_row, zrow, start=True, stop=False,
                                     skip_group_check=True)
                nc.tensor.matmul(ksum_ps, ones_row, zrow[:, 0:EI],
                                 start=True, stop=False,
                                 skip_group_check=True)

                XC = 256  # tokens per x chunk (2 tiles)
                for g in range(8):          # rstd batch group: 2 tiles
                    ku_t = {}
                    vu_t = {}
                    for cc in range(1):
                        qq = g
                        qsl = slice(XC * qq, XC * qq + XC)
                        xk_q = pax.tile([128, EI, XC], bf16, tag="xk")
                        nc.sync.dma_start(out=xk_q, in_=xkT_v[:, :, qsl])
                        if qq == 0:
                            nc.sync.dma_start(out=wk_sb[:, :, 0:512],
                                              in_=wkT_v[:, :, 0:512])
                        xv_q = pax.tile([128, EI, XC], bf16, tag="xv")
                        nc.sync.dma_start(out=xv_q, in_=xvT_v[:, :, qsl])
                        if qq == 0:
                            nc.sync.dma_start(out=wk_sb[:, :, 512:1024],
                                              in_=wkT_v[:, :, 512:1024])
                            nc.sync.dma_start(out=wv_sb, in_=wvT_v[:, :, :])
                        if qq == 2:
                            nc.sync.dma_start(out=wq_sb, in_=wqT_v[:, :, :])
                        if qq == 4:
                            nc.sync.dma_start(out=wo_sb, in_=woT_v[:, :, :])
                        if qq == 5:
                            load_xq(0)
                        if qq == 7:
                            load_xq(1)
                        for tl in range(2):
                            t = 2 * qq + tl
                            tsl = slice(128 * tl, 128 * tl + 128)
                            ku = par.tile([128, E], bf16, tag="ku", bufs=5,
                                          name=f"ku{t}")
                            for jh in range(2):
                                js = slice(512 * jh, 512 * jh + 512)
                                kps = psK.tile([128, 512], f32, tag="kps")
                                for i in range(EI):
                                    nc.tensor.matmul(kps, xk_q[:, i, tsl],
                                                     wk_sb[:, i, js],
                                                     start=(i == 0),
                                                     stop=(i == EI - 1))
                                nc.vector.scalar_tensor_tensor(
                                    out=ku[:, js], in0=kps, scalar=1.0,
                                    in1=bk_b[:, js],
                                    op0=Alu.mult, op1=Alu.add)
                            vu = par.tile([128, E], bf16, tag="vu", bufs=4,
                                          name=f"vu{t}")
                            for jh in range(2):
                                js = slice(512 * jh, 512 * jh + 512)
                                vps = psV.tile([128, 512], f32, tag="vps")
                                for i in range(EI):
                                    nc.tensor.matmul(vps, xv_q[:, i, tsl],
                                                     wv_sb[:, i, js],
                                                     start=(i == 0),
                                                     stop=(i == EI - 1))
                                nc.vector.scalar_tensor_tensor(
                                    out=vu[:, js], in0=vps, scalar=1.0,
                                    in1=bv_b[:, js],
                                    op0=Alu.mult, op1=Alu.add)
                            scrap = par.tile([128, E], bf16, tag="scrap",
                                             bufs=1)
                            nc.scalar.activation(out=scrap, in_=ku,
                                                 func=Act.Square,
                                                 accum_out=ss_all[:, t:t + 1])
                            ku_t[t] = ku
                            vu_t[t] = vu
                    # --- rstd for the 2-tile group: quake rsqrt on DVE ---
                    gs = slice(2 * g, 2 * g + 2)
                    nc.vector.tensor_scalar(
                        out=qa[:, gs], in0=ss_all[:, gs],
                        scalar1=1.0 / E, scalar2=LN_EPS,
                        op0=Alu.mult, op1=Alu.add)
                    # seed: C - (i>>1) computed as (~(i>>1)) + (C+1);
                    # bitwise ops fused together, arith add separate
                    # (walrus rejects mixed bitwise/arith fusion)
                    nc.vector.tensor_scalar(
                        out=rstd_all.bitcast(i32)[:, gs],
                        in0=qa.bitcast(i32)[:, gs],
                        scalar1=1, scalar2=-1,
                        op0=Alu.logical_shift_right, op1=Alu.bitwise_xor)
                    nc.vector.tensor_scalar(
                        out=rstd_all.bitcast(i32)[:, gs],
                        in0=rstd_all.bitcast(i32)[:, gs],
                        scalar1=QK3, scalar2=None,
                        op0=Alu.add)
                    for _ in range(2):  # Newton: y = y*(1.5 - 0.5*v*y*y)
                        nc.vector.tensor_tensor(
                            out=qc[:, gs], in0=rstd_all[:, gs],
                            in1=rstd_all[:, gs], op=Alu.mult)
                        nc.vector.scalar_tensor_tensor(
                            out=qc[:, gs], in0=qc[:, gs], scalar=-0.5,
                            in1=qa[:, gs], op0=Alu.mult, op1=Alu.mult)
                        nc.vector.scalar_tensor_tensor(
                            out=rstd_all[:, gs], in0=qc[:, gs], scalar=1.5,
                            in1=rstd_all[:, gs], op0=Alu.add, op1=Alu.mult)
                    for t in range(2 * g, 2 * g + 2):
                        ku = ku_t[t]
                        vu = vu_t[t]
                        rs = rstd_all[:, t:t + 1]
                        # elu(z)+1 = relu(z) + min(exp(z), 1)
                        km = par.tile([128, E], bf16, tag="km", bufs=2)
                        nc.scalar.activation(out=km, in_=ku, func=Act.Exp,
                                             scale=rs)
                        kf = par.tile([128, E], bf16, tag="kf", bufs=3)
                        nc.scalar.activation(out=kf, in_=ku, func=Act.Relu,
                                             scale=rs)
                        nc.vector.scalar_tensor_tensor(
                            out=kf, in0=km, scalar=1.0, in1=kf,
                            op0=Alu.min, op1=Alu.add)
                        if debug and t == 0:
                            nc.sync.dma_start(out=dbg["d_kf0"][:, :], in_=kf)
                        for j in range(EI):
                            jsl = slice(128 * j, 128 * j + 128)
                            nc.tensor.matmul(
                                vk_ps[:, jsl], vu[:, jsl], kf[:, jsl],
                                start=False, stop=(t == TT - 1),
                                skip_group_check=True)
                            nc.tensor.matmul(
                                ksum_ps[:, j:j + 1], kf[:, jsl], onesC,
                                start=False, stop=(t == TT - 1),
                                skip_group_check=True)

                # ---- pack vk diag blocks + ksum, ship to AR ----
                for h in range(16):
                    j, odd = divmod(h, 2)
                    r0 = 64 * odd
                    rsl = slice(r0, r0 + 64)
                    dst = pack[rsl, 64 * j:64 * j + 64]
                    srcp = vk_ps[rsl, 128 * j + r0:128 * j + r0 + 64]
                    if h % 2 == 0:
                        nc.vector.tensor_copy(out=dst, in_=srcp)
                    else:
                        nc.scalar.activation(out=dst, in_=srcp,
                                             func=Act.Copy)
                nc.vector.tensor_copy(out=pack[:, 512:520], in_=ksum_ps)
                nc.sync.dma_start(out=cc_in, in_=pack)
                if debug:
                    nc.sync.dma_start(out=dbg["d_pack"][:, :], in_=pack)
                    nc.sync.dma_start(out=dbg["d_rstdA"][:, :], in_=rstd_all)

            nc.gpsimd.collective_compute(
                "AllReduce", Alu.add, replica_groups=RG,
                ins=[cc_in[:, :]], outs=[cc_out[:, :]])

            def emit_elu(s2):
                # elu(z)+1 = relu(z) + min(exp(z), 1), z = u*rstd
                for j in range(EI):
                    u = u_t[j][s2]
                    nc.vector.tensor_tensor(
                        out=u, in0=u, in1=rb_t[s2], op=Alu.mult)
                    m = cp.tile([128, SC], bf16, tag="m", bufs=4)
                    nc.scalar.activation(out=m, in_=u, func=Act.Exp)
                    nc.scalar.activation(out=u, in_=u, func=Act.Relu)
                    nc.vector.scalar_tensor_tensor(
                        out=u, in0=m, scalar=1.0, in1=u,
                        op0=Alu.min, op1=Alu.add)

            # ============ Phase B1: q proj + LN stats + elu ============
            with tc.tile_pool(name="pb1", bufs=2) as pb1, \
                 tc.tile_pool(name="psB1", bufs=2, space="PSUM") as psB1:
                ln_row = pb1.tile([1, T], f32, tag="ln_row", bufs=1)
                rr16 = pb1.tile([1, T], bf16, tag="rr16", bufs=1)
                ssq_list = []
                for s in range(S):
                    if s >= 2:
                        load_xq(s)
                    xq_s = xq_tiles[s]
                    ssq_ps = psB1.tile([1, 512], f32, tag=f"ssq{s}",
                                       bufs=1, name=f"ssq{s}")
                    for j in range(EI):
                        qps = psB1.tile([128, 512], f32, tag="qps",
                                        bufs=3)
                        for i in range(EI):
                            nc.tensor.matmul(
                                qps, wq_sb[:, i, 128 * j:128 * j + 128],
                                xq_s[:, i, :], start=(i == 0),
                                stop=(i == EI - 1))
                        nc.vector.tensor_scalar_add(
                            out=u_t[j][s], in0=qps,
                            scalar1=bq_sb[:, j:j + 1])
                        usq = pb1.tile([128, 512], bf16, tag="usq")
                        nc.scalar.activation(out=usq, in_=u_t[j][s],
                                             func=Act.Square)
                        nc.tensor.matmul(ssq_ps, onesC, usq,
                                         start=(j == 0),
                                         stop=(j == EI - 1),
                                         skip_group_check=True)
                    ssq_list.append(ssq_ps)
                    if s % 2 == 1:  # batched rstd per 2 chunks
                        for s2 in (s - 1, s):
                            nc.scalar.activation(
                                out=ln_row[:, SC * s2:SC * s2 + SC],
                                in_=ssq_list[s2], func=Act.Ln,
                                scale=1.0 / E, bias=eps_row)
                        nc.scalar.activation(
                            out=rr16[:, SC * (s - 1):SC * (s + 1)],
                            in_=ln_row[:, SC * (s - 1):SC * (s + 1)],
                            func=Act.Exp, scale=-0.5)
                        for s2 in (s - 1, s):
                            ssl2 = slice(SC * s2, SC * s2 + SC)
                            rb_ps = psB1.tile([128, 512], f32, tag="rbps",
                                              bufs=1)
                            nc.tensor.matmul(rb_ps, ones_row,
                                             rr16[:, ssl2],
                                             start=True, stop=True)
                            nc.vector.tensor_copy(out=rb_t[s2], in_=rb_ps)
                        if s == 1:
                            # elu for chunks 0/1 hides under B1 chunks 2/3;
                            # chunks 2/3's elu interleaves with B2 below
                            emit_elu(0)
                            emit_elu(1)

            # ====== AR unpack (DVE; first AR-dependent ops) ======
            nc.gpsimd.dma_start(out=ar_sb, in_=cc_out[:, :])
            for h in range(16):
                j, odd = divmod(h, 2)
                r0 = 64 * odd
                rsl = slice(r0, r0 + 64)
                dstv = vkbd[rsl, 128 * j + r0:128 * j + r0 + 64]
                srcv = ar_sb[rsl, 64 * j:64 * j + 64]
                if h % 2 == 0:
                    nc.vector.tensor_copy(out=dstv, in_=srcv)
                else:
                    nc.scalar.activation(out=dstv, in_=srcv, func=Act.Copy)
            for j in range(EI):
                nc.vector.tensor_copy(
                    out=ks3[0:64, j, 2 * j:2 * j + 1],
                    in_=ar_sb[0:64, 512 + j:513 + j])
                nc.vector.tensor_copy(
                    out=ks3[64:128, j, 2 * j + 1:2 * j + 2],
                    in_=ar_sb[64:128, 512 + j:513 + j])
            if debug:
                nc.sync.dma_start(out=dbg["d_ar"][:, :], in_=ar_sb)

            # ====== M = blockdiag(vk) @ Wo^T ======
            with tc.tile_pool(name="psM", bufs=2, space="PSUM") as psM:
                for j in range(EI):
                    for jh in range(2):
                        js = slice(512 * jh, 512 * jh + 512)
                        m_ps = psM.tile([128, 512], f32, tag="mps")
                        nc.tensor.matmul(
                            m_ps, vkbd[:, 128 * j:128 * j + 128],
                            wo_sb[:, j, js], start=True, stop=True)
                        if jh == 0:
                            nc.scalar.activation(out=M_sb[:, j, js],
                                                 in_=m_ps, func=Act.Copy)
                        else:
                            nc.vector.tensor_copy(out=M_sb[:, j, js],
                                                  in_=m_ps)
            if debug:
                nc.sync.dma_start(
                    out=dbg["d_M"].rearrange("p (j c) -> p j c", c=E),
                    in_=M_sb)
                nc.sync.dma_start(out=dbg["d_rb0"][:, :], in_=rb_t[0])

            # ============ Phase B2 + C: per-chunk pipeline ============
            with tc.tile_pool(name="pc2", bufs=3) as pc2, \
                 tc.tile_pool(name="psDen", bufs=2, space="PSUM") as psDen, \
                 tc.tile_pool(name="psDrb", bufs=2, space="PSUM") as psDrb, \
                 tc.tile_pool(name="psC", bufs=3, space="PSUM") as psC:
                def emit_b2(s):
                    # pass 2: den -> 1/den (elu already done)
                    den_ps = psDen.tile([16, 512], f32, tag="dps")
                    for j in range(EI):
                        nc.tensor.matmul(den_ps, ks3[:, j, :], u_t[j][s],
                                         start=(j == 0), stop=(j == EI - 1))
                    dinv = pc2.tile([16, 512], f32, tag="dinv", bufs=2)
                    nc.vector.reciprocal_approx_fast(out=dinv, in_=den_ps)
                    dinv16 = pc2.tile([16, 512], bf16, tag="dinv16", bufs=2)
                    nc.vector.tensor_copy(out=dinv16, in_=dinv)
                    if debug and s == 0:
                        nc.sync.dma_start(out=dbg["d_dinv0"][:, :],
                                          in_=dinv16)
                        nc.sync.dma_start(out=dbg["d_u00"][:, :],
                                          in_=u_t[0][s])
                    # pass 3: broadcast 1/den to [128, SC] per block; q~ = u/den
                    for j in range(EI):
                        drb_ps = psDrb.tile([128, 512], f32, tag="drbps")
                        nc.tensor.matmul(drb_ps, ebc[:, j, :], dinv16,
                                         start=True, stop=True)
                        drb = pc2.tile([128, SC], bf16, tag="drb", bufs=4)
                        nc.vector.tensor_copy(out=drb, in_=drb_ps)
                        nc.vector.tensor_tensor(out=u_t[j][s], in0=u_t[j][s],
                                                in1=drb, op=Alu.mult)
                        if debug and s == 0 and j == 0:
                            nc.sync.dma_start(out=dbg["d_qt00"][:, :],
                                              in_=u_t[0][0])
                    # pass 4: out-proj C with M
                    for tl in range(4):
                        tsl = slice(128 * tl, 128 * tl + 128)
                        gtsl = slice(SC * s + 128 * tl,
                                     SC * s + 128 * tl + 128)
                        for jh in range(2):
                            js = slice(512 * jh, 512 * jh + 512)
                            ops = psC.tile([128, 512], f32, tag="ops")
                            for e in range(EI):
                                nc.tensor.matmul(
                                    ops, u_t[e][s][:, tsl],
                                    M_sb[:, e, js], start=(e == 0),
                                    stop=(e == EI - 1))
                            osb = pc2.tile([128, 512], f32, tag="osb", bufs=4)
                            if (tl + jh) % 2 == 0:
                                nc.scalar.activation(out=osb, in_=ops,
                                                     func=Act.Copy)
                            else:
                                nc.vector.tensor_copy(out=osb, in_=ops)
                            nc.sync.dma_start(out=out_d[gtsl, js],
                                              in_=osb)

                emit_elu(2)
                emit_b2(0)
                emit_elu(3)
                emit_b2(1)
                emit_b2(2)
                emit_b2(3)
            pbx_cm.__exit__(None, None, None)

    nc.finalize()
    return nc


def _prep_inputs(inputs):
    """Host-side fold + per-core shard maps (bf16)."""
    import ml_dtypes
    f = np.float32
    bf = ml_dtypes.bfloat16
    Wq, bq = inputs["Wq"], inputs["bq"]
    Wk, bk = inputs["Wk"], inputs["bk"]
    Wv, bv = inputs["Wv"], inputs["bv"]
    Wo = inputs["Wo"]
    for name in ("gq", "gk"):
        assert np.allclose(np.asarray(inputs[name]), 1.0), f"{name} != 1 unsupported"
    for name in ("betaq", "betak"):
        assert np.allclose(np.asarray(inputs[name]), 0.0), f"{name} != 0 unsupported"

    wqT = np.ascontiguousarray(np.asarray(Wq, f).T)
    wqT = wqT - wqT.mean(axis=1, keepdims=True)
    bqf = np.asarray(bq, f) - np.asarray(bq, f).mean()
    wkT = np.ascontiguousarray(np.asarray(Wk, f).T)
    wkT = wkT - wkT.mean(axis=1, keepdims=True)
    bkf = np.asarray(bk, f) - np.asarray(bk, f).mean()
    wvT = np.ascontiguousarray(np.asarray(Wv, f).T)
    woT = np.ascontiguousarray(np.asarray(Wo, f).T)

    ebc = np.zeros((16, EI * 128), f)
    for h in range(16):
        j, odd = divmod(h, 2)
        ebc[h, 128 * j + 64 * odd:128 * j + 64 * odd + 64] = 1.0

    shared = {
        "wqT": wqT.astype(bf),
        "wkT": wkT.astype(bf),
        "wvT": wvT.astype(bf),
        "woT": woT.astype(bf),
        "bq2d": np.ascontiguousarray(bqf.reshape(EI, 128).T, f),
        "bkR": np.ascontiguousarray(bkf.reshape(1, E), f),
        "bvR": np.ascontiguousarray(np.asarray(bv, f).reshape(1, E)),
        "ebcR": ebc.astype(bf),
    }
    qe = np.asarray(inputs["query_embed"], f)
    ke = np.asarray(inputs["key_embed"], f)
    ve = np.asarray(inputs["value"], f)
    in_maps = []
    for c in range(NCORES):
        b, hh = divmod(c, 2)
        sl = slice(hh * T, (hh + 1) * T)
        m = dict(shared)
        m["xqT"] = np.ascontiguousarray(qe[b, sl, :].T).astype(bf)
        m["xkT"] = np.ascontiguousarray(ke[b, sl, :].T).astype(bf)
        m["xvT"] = np.ascontiguousarray(ve[b, sl, :].T).astype(bf)
        in_maps.append(m)
    return in_maps


def _run(inputs, trace=False):
    from concourse.bass_utils import run_bass_kernel_spmd

    if "nc" not in _NC_CACHE:
        _NC_CACHE["nc"] = _build_nc()
    nc = _NC_CACHE["nc"]
    in_maps = _prep_inputs(inputs)
    res = run_bass_kernel_spmd(nc, in_maps, core_ids=list(range(NCORES)),
                               trace=trace)
    out = np.empty((B, NSEQ, E), np.float32)
    for c in range(NCORES):
        b, hh = divmod(c, 2)
        out[b, hh * T:(hh + 1) * T, :] = res.results[c]["out"]
    out += np.asarray(inputs["bo"], np.float32)  # bo folded on host
    return out, res


def kernel(**inputs):
    out, _ = _run(inputs, trace=False)
    return out


def kernel_traced(**inputs):
    """Like kernel() but also returns (exec_time_ns, trace_path)."""
    import sys, types
    try:
        import antenv
        if "antenv.axon_hooks" not in sys.modules:
            mod = types.ModuleType("antenv.axon_hooks")
            _h = [None]
            mod.set_axon_ntff_profile_hook = lambda h: _h.__setitem__(0, h)
            mod.get_axon_ntff_profile_hook = lambda: _h[0]
            sys.modules["antenv.axon_hooks"] = mod
            antenv.axon_hooks = mod
            from trn_agent_boot.trn_boot import _ntff_profile_via_ctypes
            mod.set_axon_ntff_profile_hook(
                _ntff_profile_via_ctypes("/opt/axon/libaxon_pjrt.so"))
    except Exception as e:  # profiling is best-effort
        print(f"NTFF hook setup failed: {e}")
    out, res = _run(inputs, trace=True)
    tp = res.instructions_and_trace[1] if res.instructions_and_trace else None
    return out, res.exec_time_ns, tp
